# revision 45
# baseline (speedup 1.0000x reference)
"""Trainium2 Bass kernel for a pre-norm transformer block (attention + MLP).

Sharding: pure data-parallel over 8 cores. Core c handles batch b=c//2 and
query-row half rh=c%2 (512 tokens). K/V are computed for the full 1024-token
batch on every core (duplicated across the pair) so no collectives are needed.

v4 (from v3): LN normalize paths rebuilt around wide bf16 DVE ops (one
sub/mult over 2048-4096 columns instead of 16 narrow fp32 ops) with fp8
copies offloaded to GpSimd; x2 residual stored in bf16 (stats matmuls go
bf16 too); b2 folded into the FC2 epilogue; FC2 runs m-outer so output DMAs
stagger; activation tables for sqrt/gelu preloaded during the o-projection
phase; input x DMAs spread over 4 queues; some attention mask-multiplies
moved to GpSimd.
"""

import numpy as np
import ml_dtypes
from contextlib import ExitStack

import concourse.bass as bass
from concourse.bacc import Bacc
import concourse.tile as tile
from concourse import mybir
from concourse.bass_utils import run_bass_kernel_spmd

F32 = mybir.dt.float32
F32R = mybir.dt.float32r
BF16 = mybir.dt.bfloat16
F8 = mybir.dt.float8e4
AF = mybir.ActivationFunctionType
ALU = mybir.AluOpType
DR = mybir.MatmulPerfMode.DoubleRow
BFNP = ml_dtypes.bfloat16
F8NP = ml_dtypes.float8_e4m3

B, N, C = 4, 1024, 1024
H, D = 16, 64
DFF = 4096
R = 512          # own query rows per core
P = 128
KC = C // P      # 8 feature k-tiles
NT = N // P      # 8 token tiles
EPS = 1e-6
WS = 32.0        # fp8 weight pre-scale
IWS = 1.0 / WS

_CACHE: dict = {}


def _bc(col_ap, n):
    """Broadcast a [128,1] column AP along the free dim to [128,n]."""
    return bass.AP(tensor=col_ap.tensor, offset=col_ap.offset,
                   ap=[col_ap.ap[0], [0, n]])


def _rep(tile_ap, times, width):
    """[128,width] AP -> [128,times,width] with the block repeated."""
    return bass.AP(tensor=tile_ap.tensor, offset=tile_ap.offset,
                   ap=[tile_ap.ap[0], [0, times], [1, width]])


def _blk(tile_ap, base, stride, nblk, width):
    """[128,?] AP -> [128,nblk,width] strided blocks starting at base."""
    sl = tile_ap[:, base:base + stride * (nblk - 1) + width]
    return bass.AP(tensor=sl.tensor, offset=sl.offset,
                   ap=[sl.ap[0], [stride, nblk], [1, width]])


def _build():
    nc = Bacc()
    io = {}
    io["xTb"] = nc.dram_tensor("xTb", [P, 2 * KC * 512], BF16, kind="ExternalInput")
    io["mskT"] = nc.dram_tensor("mskT", [N, R], BF16, kind="ExternalInput")
    io["wq8"] = nc.dram_tensor("wq8", [P, KC // 2 * 2 * C], F8, kind="ExternalInput")
    io["wk8"] = nc.dram_tensor("wk8", [P, KC // 2 * 2 * C], F8, kind="ExternalInput")
    io["w18"] = nc.dram_tensor("w18", [P, KC // 2 * 2 * DFF], F8, kind="ExternalInput")
    io["wv"] = nc.dram_tensor("wv", [C, C], BF16, kind="ExternalInput")
    io["wo"] = nc.dram_tensor("wo", [C, C], BF16, kind="ExternalInput")
    io["w2"] = nc.dram_tensor("w2", [P, DFF // P * C], BF16, kind="ExternalInput")
    for nm, n_ in [("bq", C), ("bk", C), ("bv", C), ("bo", C), ("b1", DFF), ("b2", C)]:
        io[nm] = nc.dram_tensor(nm, [n_], F32, kind="ExternalInput")
    io["sel2"] = nc.dram_tensor("sel2", [2, P], F32R, kind="ExternalInput")
    io["yT"] = nc.dram_tensor("yT", [C, R], F32, kind="ExternalOutput")

    def bias_cols(name, n_):
        return bass.AP(tensor=io[name][:].tensor, offset=0, ap=[[1, P], [P, n_ // P]])

    with tile.TileContext(nc) as tc, ExitStack() as ctx:
        # ---- long-lived sbuf pools (stack: first opened = last closed)
        const = ctx.enter_context(tc.tile_pool(name="const", bufs=1))
        xsp = ctx.enter_context(tc.tile_pool(name="xsp", bufs=4))
        x2p = ctx.enter_context(tc.tile_pool(name="x2p", bufs=1))
        yp = ctx.enter_context(tc.tile_pool(name="yp", bufs=2))
        otp = ctx.enter_context(tc.tile_pool(name="otp", bufs=KC))
        wop = ctx.enter_context(tc.tile_pool(name="wop", bufs=KC))
        vecp = ctx.enter_context(tc.tile_pool(name="vecp", bufs=4))
        bcp = ctx.enter_context(tc.tile_pool(name="bcp", bufs=2))
        t1p = ctx.enter_context(tc.tile_pool(name="t1p", bufs=2))
        sqp = ctx.enter_context(tc.tile_pool(name="sqp", bufs=2))
        xn28p = ctx.enter_context(tc.tile_pool(name="xn28p", bufs=1))

        # ---- warmup constant (no DMA dependency)
        wupt = const.tile([P, R], BF16)
        nc.vector.memset(wupt[:], 0.5)
        # preload the sqrt activation table during the initial DMA wait
        eps0 = const.tile([1, 1], F32)
        nc.vector.memset(eps0[:], EPS)
        tl0 = const.tile([1, 1], F32)
        nc.scalar.activation(tl0[:], eps0[:], AF.Sqrt)

        # ---- constants
        bq_sb = const.tile([P, C // P], F32)
        bk_sb = const.tile([P, C // P], F32)
        bo_sb = const.tile([P, C // P], F32)
        b1_sb = const.tile([P, DFF // P], F32)
        b2_sb = const.tile([P, C // P], F32)
        bv_b = const.tile([P, C], BF16)
        ones_kb = const.tile([P, 1], BF16)
        nc.vector.memset(ones_kb[:], 1.0)
        ones_cb = const.tile([1, P], BF16)
        nc.vector.memset(ones_cb[:], 1.0)
        eps_sb = const.tile([1, 1], F32)
        nc.vector.memset(eps_sb[:], EPS)
        sela = const.tile([1, P], F32R)
        selb = const.tile([1, P], F32R)

        ot = [otp.tile([P, R], BF16, tag="ot", name=f"ot{i}") for i in range(KC)]
        wo_sb = [wop.tile([P, C], BF16, tag="wo", name=f"wo{i}") for i in range(KC)]
        # x2: attn residual, bf16, one big tile (slices per m)
        x2 = x2p.tile([P, KC * R], BF16, tag="x2", name="x2")
        xn28 = xn28p.tile([P, KC * R], F8, tag="xn28", name="xn28")

        # ================= LN1 / Q / V then attention =================
        with tc.tile_pool(name="xn1p", bufs=1) as xn1p, \
             tc.tile_pool(name="xn18p", bufs=1) as xn18p, \
             tc.tile_pool(name="qtp", bufs=KC) as qtp, \
             tc.tile_pool(name="ktp", bufs=KC) as ktp, \
             tc.tile_pool(name="vtp", bufs=NT) as vtp, \
             tc.tile_pool(name="wattn", bufs=1) as wattn:
            xn1 = xn1p.tile([P, KC * N], BF16, tag="xn1", name="xn1")
            xn18 = xn18p.tile([P, KC * N], F8, tag="xn18", name="xn18")
            qt = [qtp.tile([P, R], BF16, tag="qt", name=f"qt{i}") for i in range(KC)]
            kt = [ktp.tile([P, N], BF16, tag="kt", name=f"kt{i}") for i in range(KC)]
            vt = [vtp.tile([P, H * (D + 1)], BF16, tag="vt", name=f"vt{i}")
                  for i in range(NT)]
            wk8_sb = [wattn.tile([P, 2 * C], F8, tag="wk8", name=f"wk8_{j}", bufs=4)
                      for j in range(KC // 2)]
            msk_sb = [wattn.tile([P, R], BF16, tag="msk", name=f"msk{i}", bufs=NT)
                      for i in range(NT)]

            with tc.tile_pool(name="wqvp", bufs=8) as wqvp, \
                 tc.tile_pool(name="ln_ps", bufs=2, space="PSUM") as ln_ps, \
                 tc.tile_pool(name="ln_bc", bufs=2, space="PSUM") as ln_bc, \
                 tc.tile_pool(name="mm_ps", bufs=2, space="PSUM") as mm_ps:

                # --- x (bf16) loads first on the 2 fast queues: they gate
                # everything. xbt doubles as the attn-residual source later.
                xbt = [xsp.tile([P, 4 * 512], BF16, tag="xs", name=f"xb_{i}", bufs=4)
                       for i in range(4)]
                xengs = [nc.sync, nc.scalar, nc.sync, nc.scalar]
                for i in range(4):
                    xengs[i].dma_start(out=xbt[i][:],
                                       in_=io["xTb"][:, i * 2048:(i + 1) * 2048])

                # --- then weights / masks / biases in need order
                wq8_sb = [wqvp.tile([P, 2 * C], F8, tag="wq8", name=f"wq8_{j}", bufs=4)
                          for j in range(KC // 2)]
                wv_sb = [wqvp.tile([P, C], BF16, tag="wv", name=f"wv{i}", bufs=8)
                         for i in range(KC)]
                for j in range(KC // 2):
                    nc.gpsimd.dma_start(out=wq8_sb[j][:],
                                        in_=io["wq8"][:, j * 2 * C:(j + 1) * 2 * C])
                for k in range(KC):
                    eng = nc.gpsimd if k % 2 == 0 else nc.sync
                    eng.dma_start(out=wv_sb[k][:], in_=io["wv"][k * P:(k + 1) * P, :])
                nc.sync.dma_start(out=bq_sb[:], in_=bias_cols("bq", C))
                nc.sync.dma_start(out=bk_sb[:], in_=bias_cols("bk", C))
                nc.gpsimd.dma_start(out=bv_b[:],
                                    in_=bass.AP(tensor=io["bv"][:].tensor,
                                                offset=0, ap=[[0, P], [1, C]]))
                for j in range(KC // 2):
                    nc.gpsimd.dma_start(out=wk8_sb[j][:],
                                        in_=io["wk8"][:, j * 2 * C:(j + 1) * 2 * C])
                for t in range(NT):
                    eng = nc.gpsimd if t % 2 == 0 else nc.sync
                    eng.dma_start(out=msk_sb[t][:], in_=io["mskT"][t * P:(t + 1) * P, :])
                nc.sync.dma_start(out=sela[:], in_=io["sel2"][0:1, :])
                nc.sync.dma_start(out=selb[:], in_=io["sel2"][1:2, :])
                nc.sync.dma_start(out=bo_sb[:], in_=bias_cols("bo", C))
                nc.sync.dma_start(out=b1_sb[:], in_=bias_cols("b1", DFF))
                nc.sync.dma_start(out=b2_sb[:], in_=bias_cols("b2", C))

                wup_ct = [0]

                def warm(n_mm):
                    wup = mm_ps.tile([P, 512], F32, tag="mm",
                                     name=f"wup{wup_ct[0]}")
                    for i in range(n_mm):
                        nc.tensor.matmul(wup[:], wupt[:, 0:P], wupt[:],
                                         start=(i == 0), stop=(i == n_mm - 1))
                    ws_ = vecp.tile([1, 1], F32, tag="vec", name=f"wups{wup_ct[0]}")
                    nc.scalar.copy(ws_[:], wup[0:1, 0:1])
                    wup_ct[0] += 1

                warm(20)

                # --- LN1 over 2 chunks (bf16 stats via ones-matmul reductions)
                # all four stat rows share ONE psum bank: [s0|q0|s1|q1]
                inv_c = 1.0 / C
                ps_st0 = ln_ps.tile([1, 2 * 512], F32, tag="lnstat", name="ln1st0")
                ps_st1 = ln_ps.tile([1, 2 * 512], F32, tag="lnstat", name="ln1st1")

                def ln_chain(ps_s, ps_q, tag):
                    """[1,512] psum sums -> mu_bc/rstd_bc [P,512] bf16 SBUF."""
                    mu = vecp.tile([1, 512], F32, tag="vec", name=f"mu{tag}")
                    nc.vector.tensor_scalar_mul(mu[:], ps_s, inv_c)
                    msq = vecp.tile([1, 512], F32, tag="vec", name=f"msq{tag}")
                    nc.vector.tensor_mul(msq[:], mu[:], mu[:])
                    varr = vecp.tile([1, 512], F32, tag="vec", name=f"var{tag}")
                    nc.vector.scalar_tensor_tensor(varr[:], ps_q, inv_c, msq[:],
                                                   op0=ALU.mult, op1=ALU.subtract)
                    std = vecp.tile([1, 512], F32, tag="vec", name=f"std{tag}")
                    nc.scalar.activation(std[:], varr[:], AF.Sqrt, bias=eps_sb[:])
                    rstd = vecp.tile([1, 512], F32, tag="vec", name=f"rstd{tag}")
                    nc.vector.reciprocal_approx_fast(out=rstd[:], in_=std[:])
                    mu_b = vecp.tile([1, 512], BF16, tag="vb16", name=f"mub{tag}", bufs=2)
                    rstd_b = vecp.tile([1, 512], BF16, tag="vb16", name=f"rsb{tag}", bufs=2)
                    nc.scalar.copy(mu_b[:], mu[:])
                    nc.vector.tensor_copy(out=rstd_b[:], in_=rstd[:])
                    return mu_b, rstd_b

                def ln_bcast(mu_b, rstd_b, tag):
                    ps_mu = ln_bc.tile([P, 512], F32, tag="lnbc", name=f"bmu{tag}")
                    ps_rstd = ln_bc.tile([P, 512], F32, tag="lnbc", name=f"brs{tag}")
                    nc.tensor.matmul(ps_mu[:], ones_cb[:], mu_b[:], start=True, stop=True)
                    nc.tensor.matmul(ps_rstd[:], ones_cb[:], rstd_b[:], start=True, stop=True)
                    mu_bc = bcp.tile([P, 512], BF16, tag="bc", name=f"mubc{tag}", bufs=2)
                    rstd_bc = bcp.tile([P, 512], BF16, tag="bc", name=f"rsbc{tag}", bufs=2)
                    nc.vector.tensor_copy(out=mu_bc[:], in_=ps_mu[:])
                    nc.vector.tensor_copy(out=rstd_bc[:], in_=ps_rstd[:])
                    return mu_bc, rstd_bc

                def ln1_norm(chunk, mu_bc, rstd_bc):
                    # chunk-block layout keeps writes contiguous:
                    # col = chunk*4096 + k*512 + tok
                    for half in range(2):
                        xb = xbt[chunk * 2 + half]
                        t1 = t1p.tile([P, 2048], BF16, tag="t1",
                                      name=f"t1_{chunk}_{half}")
                        nc.vector.tensor_tensor(t1[:].rearrange("p (b w) -> p b w", w=512),
                                                xb[:].rearrange("p (b w) -> p b w", w=512),
                                                _rep(mu_bc[:], 4, 512), op=ALU.subtract)
                        obase = chunk * 4096 + half * 2048
                        dst = xn1[:, obase:obase + 2048]
                        nc.vector.tensor_tensor(
                            dst.rearrange("p (b w) -> p b w", w=512),
                            t1[:].rearrange("p (b w) -> p b w", w=512),
                            _rep(rstd_bc[:], 4, 512), op=ALU.mult)
                        # fp8 copy for Q/K DoubleRow rhs (contiguous, on scalar)
                        nc.scalar.copy(xn18[:, obase:obase + 2048], dst)

                # stats for BOTH chunks back-to-back on PE
                for chunk in range(2):
                    sqc = [None, None]
                    for half in range(2):
                        xb = xbt[chunk * 2 + half]
                        sqc[half] = sqp.tile([P, 2048], BF16, tag="sq",
                                             name=f"sq1_{chunk}_{half}", bufs=2)
                        nc.vector.tensor_mul(sqc[half][:], xb[:], xb[:])
                    ps_c = ps_st0 if chunk == 0 else ps_st1
                    ps_s = ps_c[:, 0:512]
                    ps_q = ps_c[:, 512:1024]
                    for k in range(KC):
                        xc = xbt[chunk * 2 + k // 4][:, (k % 4) * 512:(k % 4 + 1) * 512]
                        sq = sqc[k // 4][:, (k % 4) * 512:(k % 4 + 1) * 512]
                        nc.tensor.matmul(ps_s, ones_kb[:], xc,
                                         start=(k == 0), stop=(k == KC - 1))
                        nc.tensor.matmul(ps_q, ones_kb[:], sq,
                                         start=(k == 0), stop=(k == KC - 1))
                warm(18)
                mu_b0, rstd_b0 = ln_chain(ps_st0[:, 0:512], ps_st0[:, 512:1024], "c0")
                mu_bc0, rstd_bc0 = ln_bcast(mu_b0, rstd_b0, "c0")
                warm(18)
                ln1_norm(0, mu_bc0, rstd_bc0)

                # ---- V projection first two token tiles (needs only xn1 chunk 0)
                def emit_v(t):
                    vre = vt[t][:].rearrange("p (h j) -> p h j", j=D + 1)
                    nc.vector.memset(vre[:, :, D:D + 1], 1.0)
                    for half in range(2):
                        ps = mm_ps.tile([P, 512], F32, tag="mm")
                        for k in range(KC):
                            xb = (t // 4) * 4096 + k * 512 + (t % 4) * P
                            nc.tensor.matmul(ps[:], xn1[:, xb:xb + P],
                                             wv_sb[k][:, half * 512:(half + 1) * 512],
                                             start=(k == 0), stop=(k == KC - 1))
                        nc.vector.tensor_tensor(
                            vre[:, 8 * half:8 * (half + 1), 0:D],
                            ps[:].rearrange("p (h j) -> p h j", j=D),
                            bv_b[:, half * 512:(half + 1) * 512].rearrange(
                                "p (h j) -> p h j", j=D),
                            op=ALU.add)

                emit_v(0)
                emit_v(1)
                emit_v(2)
                emit_v(3)

                # chunk-1 chain overlaps V0-3 PE work; its broadcast matmuls
                # land in the PE FIFO after V0-3
                mu_b1, rstd_b1 = ln_chain(ps_st1[:, 0:512], ps_st1[:, 512:1024], "c1")
                mu_bc1, rstd_bc1 = ln_bcast(mu_b1, rstd_b1, "c1")

                # ---- Q projection (own tokens only), fp8 DoubleRow
                for m in range(KC):
                    ps = mm_ps.tile([P, 512], F32, tag="mm")
                    for j in range(KC // 2):
                        w8r = wq8_sb[j][:].rearrange("p (t m) -> p t m", t=2)
                        x8r = xn18[:, j * 1024:(j + 1) * 1024].rearrange(
                            "p (t n) -> p t n", t=2)
                        nc.tensor.matmul(ps[:], w8r[:, :, m * P:(m + 1) * P],
                                         x8r[:],
                                         start=(j == 0), stop=(j == KC // 2 - 1),
                                         perf_mode=DR)
                    nc.scalar.activation(qt[m][:], ps[:], AF.Identity,
                                         scale=IWS * float(D) ** -0.5,
                                         bias=bq_sb[:, m:m + 1])

                ln1_norm(1, mu_bc1, rstd_bc1)
                for t in range(4, NT):
                    emit_v(t)

            # ---- attention pair pipeline (K projection pipelined one ahead)
            with tc.tile_pool(name="simps", bufs=2, space="PSUM") as simps, \
                 tc.tile_pool(name="ops", bufs=1, space="PSUM") as ops_, \
                 tc.tile_pool(name="mmb", bufs=2, space="PSUM") as mmb, \
                 tc.tile_pool(name="rbps", bufs=1, space="PSUM") as rbps, \
                 tc.tile_pool(name="a2p", bufs=10) as a2p, \
                 tc.tile_pool(name="recp", bufs=2) as recp, \
                 tc.tile_pool(name="smr", bufs=2) as smr:

                # prefetch wo during attention (gpsimd queue — sync is busy)
                for k in range(KC):
                    nc.gpsimd.dma_start(out=wo_sb[k][:], in_=io["wo"][k * P:(k + 1) * P, :])

                a_tiles = {}
                sums = {}

                def emit_k(p):
                    for nn_ in range(2):
                        ps = mmb.tile([P, 512], F32, tag="mm", name=f"kp{p}_{nn_}")
                        for j in range(KC // 2):
                            w8r = wk8_sb[j][:].rearrange("p (t m) -> p t m", t=2)
                            xb = nn_ * 4096 + j * 1024
                            x8r = xn18[:, xb:xb + 1024].rearrange(
                                "p (t n) -> p t n", t=2)
                            nc.tensor.matmul(ps[:], w8r[:, :, p * P:(p + 1) * P],
                                             x8r[:],
                                             start=(j == 0), stop=(j == KC // 2 - 1),
                                             perf_mode=DR)
                        nc.vector.scalar_tensor_tensor(
                            kt[p][:, nn_ * 512:(nn_ + 1) * 512], ps[:], IWS,
                            _bc(bk_sb[:, p:p + 1], 512), op0=ALU.mult, op1=ALU.add)

                def emit_qk(p):
                    kth0 = kt[p][0:D, :]
                    kth1 = kt[p][D:2 * D, :]
                    qth0 = qt[p][0:D, :]
                    qth1 = qt[p][D:2 * D, :]
                    for tk in range(NT):
                        ps2 = simps.tile([P, 2 * R], F32, tag="sim", name=f"sim{p}_{tk}")
                        nc.tensor.matmul(ps2[:, 0:R], kth0[:, tk * P:(tk + 1) * P], qth0[:],
                                         start=True, stop=True)
                        nc.tensor.matmul(ps2[:, R:2 * R], kth1[:, tk * P:(tk + 1) * P], qth1[:],
                                         start=True, stop=True)
                        a2 = a2p.tile([P, 2 * R], BF16, tag="a", name=f"a{p}_{tk}")
                        nc.scalar.activation(a2[:], ps2[:], AF.Exp)
                        mrep = bass.AP(tensor=msk_sb[tk][:].tensor,
                                       offset=msk_sb[tk][:].offset,
                                       ap=[msk_sb[tk][:].ap[0], [0, 2], [1, R]])
                        nc.vector.tensor_tensor(
                            a2[:].rearrange("p (h j) -> p h j", j=R),
                            a2[:].rearrange("p (h j) -> p h j", j=R), mrep, op=ALU.mult)
                        a_tiles[(p, tk)] = a2

                def emit_o(p):
                    s0 = smr.tile([1, R], F32R, tag="s0", name=f"s0_{p}")
                    s1 = smr.tile([1, R], F32R, tag="s1", name=f"s1_{p}")
                    sums[p] = (s0, s1)
                    for hh in range(2):
                        h = 2 * p + hh
                        ps_o = ops_.tile([D + 1, R], F32, tag="o", name=f"o{h}")
                        for tk in range(NT):
                            vre = vt[tk][:].rearrange("p (h j) -> p h j", j=D + 1)
                            nc.tensor.matmul(ps_o[:], vre[:, h, 0:D + 1],
                                             a_tiles[(p, tk)][:, hh * R:(hh + 1) * R],
                                             start=(tk == 0), stop=(tk == NT - 1))
                        dst = s0 if hh == 0 else s1
                        nc.vector.tensor_copy(out=dst[0:1, :], in_=ps_o[D:D + 1, :])
                        nc.vector.tensor_copy(out=ot[p][hh * D:(hh + 1) * D, :],
                                              in_=ps_o[0:D, :])
                    for tk in range(NT):
                        del a_tiles[(p, tk)]

                def emit_norm(p):
                    s0, s1 = sums.pop(p)
                    ps_rb = rbps.tile([P, R], F32, tag="rb", name=f"rb{p}")
                    nc.tensor.matmul(ps_rb[:], sela[:], s0[0:1, :],
                                     start=True, stop=False)
                    nc.tensor.matmul(ps_rb[:], selb[:], s1[0:1, :],
                                     start=False, stop=True)
                    rec_sb = recp.tile([P, R], F32, tag="rec", name=f"rec{p}")
                    nc.vector.reciprocal_approx_fast(out=rec_sb[:], in_=ps_rb[:])
                    nc.vector.tensor_tensor(ot[p][:], ot[p][:], rec_sb[:], op=ALU.mult)

                emit_k(0)
                for p in range(H // 2):
                    emit_qk(p)
                    if p < H // 2 - 1:
                        emit_k(p + 1)
                    emit_o(p)
                    emit_norm(p)

                # preload sqrt+gelu activation tables right after the last exp
                # (input dep on ot[7] pins these behind the attention tail)
                tl1 = vecp.tile([1, 1], F32, tag="vec", name="tl1")
                tl2 = vecp.tile([1, 1], F32, tag="vec", name="tl2")
                nc.scalar.activation(tl1[:], ot[7][0:1, 0:1], AF.Sqrt)
                nc.scalar.activation(tl2[:], tl1[:], AF.Gelu_apprx_tanh)

        # ================= attn out projection + residual + LN2 stats =================
        # w18 loads during this phase into a late-opened pool
        w18p = ctx.enter_context(tc.tile_pool(name="w18p", bufs=KC // 2))
        w18_sb = [w18p.tile([P, 2 * DFF], F8, tag="w18", name=f"w18_{j}")
                  for j in range(KC // 2)]
        for j in range(KC // 2):
            eng = nc.sync if j % 2 == 0 else nc.gpsimd
            eng.dma_start(out=w18_sb[j][:],
                          in_=io["w18"][:, j * 2 * DFF:(j + 1) * 2 * DFF])

        inv_c = 1.0 / C
        with tc.tile_pool(name="ln_ps2", bufs=2, space="PSUM") as ln_ps, \
             tc.tile_pool(name="ln_bc2", bufs=2, space="PSUM") as ln_bc, \
             tc.tile_pool(name="mm_ps2", bufs=3, space="PSUM") as mm_ps:
            ps_s = ln_ps.tile([1, 512], F32, tag="lnstat", name="ln2s")
            ps_q = ln_ps.tile([1, 512], F32, tag="lnstat", name="ln2q")
            sq2h = [sqp.tile([P, 4 * R], BF16, tag="sq", name=f"sq2h{i}", bufs=2)
                    for i in range(2)]

            def oproj(m):
                ps = mm_ps.tile([P, 512], F32, tag="mm")
                for k in range(KC):
                    nc.tensor.matmul(ps[:], wo_sb[k][:, m * P:(m + 1) * P], ot[k][:],
                                     start=(k == 0), stop=(k == KC - 1))
                # residual x (bf16) is still resident in the xbt input tiles
                xr = xbt[m // 4][:, (m % 4) * 512:(m % 4 + 1) * 512]
                nc.vector.scalar_tensor_tensor(x2[:, m * R:(m + 1) * R], ps[:],
                                               bo_sb[:, m:m + 1], xr,
                                               op0=ALU.add, op1=ALU.add)

            def ln2_stats(k):
                nc.tensor.matmul(ps_s[:], ones_kb[:], x2[:, k * R:(k + 1) * R],
                                 start=(k == 0), stop=(k == KC - 1))
                nc.tensor.matmul(ps_q[:], ones_kb[:],
                                 sq2h[k // 4][:, (k % 4) * R:(k % 4 + 1) * R],
                                 start=(k == 0), stop=(k == KC - 1))

            wup_ct2 = [0]

            def warm2(n_mm):
                wup = ln_bc.tile([P, 512], F32, tag="lnbc", name=f"w2u{wup_ct2[0]}")
                for i in range(n_mm):
                    nc.tensor.matmul(wup[:], wupt[:, 0:P], wupt[:],
                                     start=(i == 0), stop=(i == n_mm - 1))
                wsb = vecp.tile([1, 1], F32, tag="vec", name=f"w2us{wup_ct2[0]}")
                nc.scalar.copy(wsb[:], wup[0:1, 0:1])
                wup_ct2[0] += 1

            for m in range(4):
                oproj(m)
            nc.vector.tensor_mul(sq2h[0][:], x2[:, 0:4 * R], x2[:, 0:4 * R])
            for m in range(4, KC):
                oproj(m)
                ln2_stats(m - 4)
            nc.vector.tensor_mul(sq2h[1][:], x2[:, 4 * R:8 * R], x2[:, 4 * R:8 * R])
            warm2(16)
            for k in range(4, KC):
                ln2_stats(k)
            warm2(14)

            mu_bc, rstd_bc = None, None
            mu = vecp.tile([1, 512], F32, tag="vec", name="mu_l2")
            nc.vector.tensor_scalar_mul(mu[:], ps_s[:], inv_c)
            msq = vecp.tile([1, 512], F32, tag="vec", name="msq_l2")
            nc.vector.tensor_mul(msq[:], mu[:], mu[:])
            varr = vecp.tile([1, 512], F32, tag="vec", name="var_l2")
            nc.vector.scalar_tensor_tensor(varr[:], ps_q[:], inv_c, msq[:],
                                           op0=ALU.mult, op1=ALU.subtract)
            std = vecp.tile([1, 512], F32, tag="vec", name="std_l2")
            nc.scalar.activation(std[:], varr[:], AF.Sqrt, bias=eps_sb[:])
            rstd = vecp.tile([1, 512], F32, tag="vec", name="rstd_l2")
            nc.vector.reciprocal_approx_fast(out=rstd[:], in_=std[:])
            mu_b = vecp.tile([1, 512], BF16, tag="vb16", name="mub_l2", bufs=2)
            rstd_b = vecp.tile([1, 512], BF16, tag="vb16", name="rsb_l2", bufs=2)
            nc.gpsimd.tensor_copy(out=mu_b[:], in_=mu[:])
            nc.vector.tensor_copy(out=rstd_b[:], in_=rstd[:])
            ps_mu = ln_bc.tile([P, 512], F32, tag="lnbc", name="bmu_l2")
            ps_rstd = ln_bc.tile([P, 512], F32, tag="lnbc", name="brs_l2")
            nc.tensor.matmul(ps_mu[:], ones_cb[:], mu_b[:], start=True, stop=True)
            nc.tensor.matmul(ps_rstd[:], ones_cb[:], rstd_b[:], start=True, stop=True)
            mu_bc = bcp.tile([P, 512], BF16, tag="bc", name="mubc_l2", bufs=2)
            rstd_bc = bcp.tile([P, 512], BF16, tag="bc", name="rsbc_l2", bufs=2)
            nc.vector.tensor_copy(out=mu_bc[:], in_=ps_mu[:])
            nc.vector.tensor_copy(out=rstd_bc[:], in_=ps_rstd[:])
            # keep PE warm while the normalize runs on the vector engine
            warm2(12)
            # normalize in two wide bf16 halves -> fp8 (k-contiguous layout)
            for hf in range(2):
                t1 = t1p.tile([P, 4 * R], BF16, tag="t1", name=f"t1l2_{hf}")
                nc.vector.tensor_tensor(
                    t1[:].rearrange("p (b w) -> p b w", w=R),
                    x2[:, hf * 4 * R:(hf + 1) * 4 * R].rearrange("p (b w) -> p b w", w=R),
                    _rep(mu_bc[:], 4, R), op=ALU.subtract)
                nc.vector.tensor_tensor(
                    xn28[:, hf * 4 * R:(hf + 1) * 4 * R].rearrange("p (b w) -> p b w", w=R),
                    t1[:].rearrange("p (b w) -> p b w", w=R),
                    _rep(rstd_bc[:], 4, R), op=ALU.mult)

        # ================= MLP =================
        h1p = ctx.enter_context(tc.tile_pool(name="h1p", bufs=DFF // P))
        h1 = [h1p.tile([P, R], BF16, tag="h1", name=f"h1_{i}") for i in range(DFF // P)]
        with tc.tile_pool(name="w2p", bufs=6) as w2p, \
             tc.tile_pool(name="mm_ps3", bufs=3, space="PSUM") as mm_ps, \
             tc.tile_pool(name="fc2ps", bufs=2, space="PSUM") as fc2ps:
            # w2 packed as (mg, k) blocks of [128, 512]; 8 DMAs of [128, 4096]
            w2_sb = [w2p.tile([P, 4096], BF16, tag="w2s", name=f"w2_{g}", bufs=6)
                     for g in range(KC)]
            for g in range(KC):
                nc.gpsimd.dma_start(out=w2_sb[g][:],
                                    in_=io["w2"][:, g * 4096:(g + 1) * 4096])

            def w2ap(mg, k, om):
                g = mg * 4 + k // 8
                return w2_sb[g][:, (k % 8) * 512 + om * P:(k % 8) * 512 + (om + 1) * P]

            # fc1 (fp8 DoubleRow) + gelu
            for m in range(DFF // P):
                ps = mm_ps.tile([P, 512], F32, tag="mm")
                for j in range(KC // 2):
                    w8r = w18_sb[j][:].rearrange("p (t m) -> p t m", t=2)
                    x8r = xn28[:, j * 2 * R:(j + 1) * 2 * R].rearrange(
                        "p (t n) -> p t n", t=2)
                    nc.tensor.matmul(ps[:], w8r[:, :, m * P:(m + 1) * P], x8r,
                                     start=(j == 0), stop=(j == KC // 2 - 1),
                                     perf_mode=DR)
                nc.scalar.activation(h1[m][:], ps[:], AF.Gelu_apprx_tanh, scale=IWS,
                                     bias=b1_sb[:, m:m + 1])

            # fc2 (bf16) m-outer: epilogue + output DMA staggered per m-tile
            for om in range(KC):
                ps = fc2ps.tile([P, 512], F32, tag="fc2", name=f"fc2ps{om}")
                for k in range(DFF // P):
                    nc.tensor.matmul(ps[:], w2ap(om // 4, k, om % 4),
                                     h1[k][:], start=(k == 0),
                                     stop=(k == DFF // P - 1))
                y_sb = yp.tile([P, R], F32, tag="y", name=f"y{om}")
                nc.vector.scalar_tensor_tensor(y_sb[:], ps[:], b2_sb[:, om:om + 1],
                                               x2[:, om * R:(om + 1) * R],
                                               op0=ALU.add, op1=ALU.add)
                eng = nc.sync if om % 2 == 0 else nc.gpsimd
                eng.dma_start(out=io["yT"][om * P:(om + 1) * P, :], in_=y_sb[:])

    if not nc.is_finalized():
        nc.finalize()
    return nc


def _get_nc():
    if "nc" not in _CACHE:
        _CACHE["nc"] = _build()
    return _CACHE["nc"]


def _pack_pairs(w):
    """[K, M] -> [128, (K/256)*2*M]: per pair j, [W_{2j} | W_{2j+1}]."""
    K_, M_ = w.shape
    nj = K_ // (2 * P)
    out = np.empty((P, nj * 2 * M_), dtype=w.dtype)
    for j in range(nj):
        out[:, j * 2 * M_:j * 2 * M_ + M_] = w[2 * j * P:(2 * j + 1) * P, :]
        out[:, j * 2 * M_ + M_:(j + 1) * 2 * M_] = w[(2 * j + 1) * P:(2 * j + 2) * P, :]
    return out


def _prep_in_maps(inputs):
    x = np.asarray(inputs["x"], dtype=np.float32)
    mask = np.asarray(inputs["mask"])
    scale = float(D) ** -0.5
    wq = np.asarray(inputs["wq"], np.float32)
    bq = (np.asarray(inputs["bq"], np.float32) * scale).astype(np.float32)
    wkv = np.asarray(inputs["wkv"], np.float32)
    bkv = np.asarray(inputs["bkv"], np.float32)
    wk = np.ascontiguousarray(wkv[:, :C])
    wv = np.ascontiguousarray(wkv[:, C:]).astype(BFNP)
    bk = np.ascontiguousarray(bkv[:C]).astype(np.float32)
    bv = np.ascontiguousarray(bkv[C:]).astype(np.float32)
    wo = np.asarray(inputs["wo"], np.float32).astype(BFNP)
    bo = np.asarray(inputs["bo"], np.float32)
    w1 = np.asarray(inputs["w1"], np.float32)
    b1 = np.asarray(inputs["b1"], np.float32)
    w2r = np.asarray(inputs["w2"], np.float32).astype(BFNP)
    # pack w2 as (mg, k) blocks: w2pk[p, mg*16384 + k*512 + j] = w2[k*128+p, mg*512+j]
    w2 = np.ascontiguousarray(
        w2r.reshape(DFF // P, P, 2, 512).transpose(1, 2, 0, 3).reshape(P, DFF // P * C))
    b2 = np.asarray(inputs["b2"], np.float32)
    mask01 = mask.astype(np.float32)

    # fp8 weights, pre-scaled by WS (epilogues rescale by 1/WS); q additionally
    # folds d^-0.5 into its activation scale, and bq is pre-scaled by d^-0.5
    wq8 = _pack_pairs(np.clip(wq * WS, -240, 240).astype(F8NP))
    wk8 = _pack_pairs(np.clip(wk * WS, -240, 240).astype(F8NP))
    w18 = _pack_pairs(np.clip(w1 * WS, -240, 240).astype(F8NP))

    sel2 = np.zeros((2, P), dtype=np.float32)
    sel2[0, 0:D] = 1.0
    sel2[1, D:2 * D] = 1.0
    shared = dict(wq8=wq8, wk8=wk8, w18=w18, w2=w2, wv=wv, wo=wo,
                  bq=bq, bk=bk, bv=bv, bo=bo, b1=b1, b2=b2, sel2=sel2)
    in_maps = []
    for c in range(8):
        b = c // 2
        rh = c % 2
        own = np.arange(rh * R, rh * R + R)
        oth = np.arange((1 - rh) * R, (1 - rh) * R + R)
        perm = np.concatenate([own, oth])
        xT = np.ascontiguousarray(x[b].T[:, perm])
        mskT = np.ascontiguousarray(mask01[np.ix_(own, perm)].T).astype(BFNP)
        m = dict(shared)
        # pack x (bf16): blocks ordered (chunk, k) of [128, 512]
        xb = xT.astype(BFNP).reshape(KC, P, 2, 512).transpose(1, 2, 0, 3).reshape(P, 2 * KC * 512)
        m["xTb"] = np.ascontiguousarray(xb)
        m["mskT"] = mskT
        in_maps.append(m)
    return in_maps


def _assemble(results):
    out = np.empty((B, N, C), dtype=np.float32)
    for c in range(8):
        b = c // 2
        rh = c % 2
        out[b, rh * R:(rh + 1) * R, :] = results[c]["yT"].T
    return out


def run(inputs, trace=False):
    nc = _get_nc()
    in_maps = _prep_in_maps(inputs)
    res = run_bass_kernel_spmd(nc, in_maps, core_ids=list(range(8)), trace=trace)
    return _assemble(res.results), res


def kernel(**inputs):
    out, _ = run(inputs, trace=False)
    return out


# revision 47
# speedup vs baseline: 1.0096x; 1.0096x over previous
"""Trainium2 Bass kernel for a pre-norm transformer block (attention + MLP).

Sharding: pure data-parallel over 8 cores. Core c handles batch b=c//2 and
query-row half rh=c%2 (512 tokens). K/V are computed for the full 1024-token
batch on every core (duplicated across the pair) so no collectives are needed.

v4 (from v3): LN normalize paths rebuilt around wide bf16 DVE ops (one
sub/mult over 2048-4096 columns instead of 16 narrow fp32 ops) with fp8
copies offloaded to GpSimd; x2 residual stored in bf16 (stats matmuls go
bf16 too); b2 folded into the FC2 epilogue; FC2 runs m-outer so output DMAs
stagger; activation tables for sqrt/gelu preloaded during the o-projection
phase; input x DMAs spread over 4 queues; some attention mask-multiplies
moved to GpSimd.
"""

import numpy as np
import ml_dtypes
from contextlib import ExitStack

import concourse.bass as bass
from concourse.bacc import Bacc
import concourse.tile as tile
from concourse import mybir
from concourse.bass_utils import run_bass_kernel_spmd

F32 = mybir.dt.float32
F32R = mybir.dt.float32r
BF16 = mybir.dt.bfloat16
F8 = mybir.dt.float8e4
AF = mybir.ActivationFunctionType
ALU = mybir.AluOpType
DR = mybir.MatmulPerfMode.DoubleRow
BFNP = ml_dtypes.bfloat16
F8NP = ml_dtypes.float8_e4m3

B, N, C = 4, 1024, 1024
H, D = 16, 64
DFF = 4096
R = 512          # own query rows per core
P = 128
KC = C // P      # 8 feature k-tiles
NT = N // P      # 8 token tiles
EPS = 1e-6
WS = 32.0        # fp8 weight pre-scale
IWS = 1.0 / WS

_CACHE: dict = {}


def _bc(col_ap, n):
    """Broadcast a [128,1] column AP along the free dim to [128,n]."""
    return bass.AP(tensor=col_ap.tensor, offset=col_ap.offset,
                   ap=[col_ap.ap[0], [0, n]])


def _rep(tile_ap, times, width):
    """[128,width] AP -> [128,times,width] with the block repeated."""
    return bass.AP(tensor=tile_ap.tensor, offset=tile_ap.offset,
                   ap=[tile_ap.ap[0], [0, times], [1, width]])


def _blk(tile_ap, base, stride, nblk, width):
    """[128,?] AP -> [128,nblk,width] strided blocks starting at base."""
    sl = tile_ap[:, base:base + stride * (nblk - 1) + width]
    return bass.AP(tensor=sl.tensor, offset=sl.offset,
                   ap=[sl.ap[0], [stride, nblk], [1, width]])


def _build():
    nc = Bacc()
    io = {}
    io["xTb"] = nc.dram_tensor("xTb", [P, 2 * KC * 512], BF16, kind="ExternalInput")
    io["mskT"] = nc.dram_tensor("mskT", [N, R], BF16, kind="ExternalInput")
    io["wq8"] = nc.dram_tensor("wq8", [P, KC // 2 * 2 * C], F8, kind="ExternalInput")
    io["wk8"] = nc.dram_tensor("wk8", [P, KC // 2 * 2 * C], F8, kind="ExternalInput")
    io["w18"] = nc.dram_tensor("w18", [P, KC // 2 * 2 * DFF], F8, kind="ExternalInput")
    io["wv"] = nc.dram_tensor("wv", [C, C], BF16, kind="ExternalInput")
    io["wo"] = nc.dram_tensor("wo", [C, C], BF16, kind="ExternalInput")
    io["w2"] = nc.dram_tensor("w2", [P, DFF // P * C], BF16, kind="ExternalInput")
    for nm, n_ in [("bq", C), ("bk", C), ("bv", C), ("bo", C), ("b1", DFF), ("b2", C)]:
        io[nm] = nc.dram_tensor(nm, [n_], F32, kind="ExternalInput")
    io["sel2"] = nc.dram_tensor("sel2", [2, P], F32R, kind="ExternalInput")
    io["yT"] = nc.dram_tensor("yT", [C, R], F32, kind="ExternalOutput")

    def bias_cols(name, n_):
        return bass.AP(tensor=io[name][:].tensor, offset=0, ap=[[1, P], [P, n_ // P]])

    with tile.TileContext(nc) as tc, ExitStack() as ctx:
        # ---- long-lived sbuf pools (stack: first opened = last closed)
        const = ctx.enter_context(tc.tile_pool(name="const", bufs=1))
        xsp = ctx.enter_context(tc.tile_pool(name="xsp", bufs=4))
        x2p = ctx.enter_context(tc.tile_pool(name="x2p", bufs=1))
        yp = ctx.enter_context(tc.tile_pool(name="yp", bufs=2))
        otp = ctx.enter_context(tc.tile_pool(name="otp", bufs=KC))
        wop = ctx.enter_context(tc.tile_pool(name="wop", bufs=KC))
        vecp = ctx.enter_context(tc.tile_pool(name="vecp", bufs=4))
        bcp = ctx.enter_context(tc.tile_pool(name="bcp", bufs=2))
        t1p = ctx.enter_context(tc.tile_pool(name="t1p", bufs=2))
        sqp = ctx.enter_context(tc.tile_pool(name="sqp", bufs=2))
        xn28p = ctx.enter_context(tc.tile_pool(name="xn28p", bufs=1))

        # ---- warmup constant (no DMA dependency)
        wupt = const.tile([P, R], BF16)
        nc.vector.memset(wupt[:], 0.5)
        # preload the sqrt activation table during the initial DMA wait
        eps0 = const.tile([1, 1], F32)
        nc.vector.memset(eps0[:], EPS)
        tl0 = const.tile([1, 1], F32)
        nc.scalar.activation(tl0[:], eps0[:], AF.Sqrt)

        # ---- constants
        bq_sb = const.tile([P, C // P], F32)
        bk_sb = const.tile([P, C // P], F32)
        bo_sb = const.tile([P, C // P], F32)
        b1_sb = const.tile([P, DFF // P], F32)
        b2_sb = const.tile([P, C // P], F32)
        bv_b = const.tile([P, C], BF16)
        ones_kb = const.tile([P, 1], BF16)
        nc.vector.memset(ones_kb[:], 1.0)
        ones_cb = const.tile([1, P], BF16)
        nc.vector.memset(ones_cb[:], 1.0)
        eps_sb = const.tile([1, 1], F32)
        nc.vector.memset(eps_sb[:], EPS)
        sela = const.tile([1, P], F32R)
        selb = const.tile([1, P], F32R)

        ot = [otp.tile([P, R], BF16, tag="ot", name=f"ot{i}") for i in range(KC)]
        wo_sb = [wop.tile([P, C], BF16, tag="wo", name=f"wo{i}") for i in range(KC)]
        # x2: attn residual, bf16, one big tile (slices per m)
        x2 = x2p.tile([P, KC * R], BF16, tag="x2", name="x2")
        xn28 = xn28p.tile([P, KC * R], F8, tag="xn28", name="xn28")

        # ================= LN1 / Q / V then attention =================
        with tc.tile_pool(name="xn1p", bufs=1) as xn1p, \
             tc.tile_pool(name="xn18p", bufs=1) as xn18p, \
             tc.tile_pool(name="qtp", bufs=KC) as qtp, \
             tc.tile_pool(name="ktp", bufs=KC) as ktp, \
             tc.tile_pool(name="vtp", bufs=NT) as vtp, \
             tc.tile_pool(name="wattn", bufs=1) as wattn:
            xn1 = xn1p.tile([P, KC * N], BF16, tag="xn1", name="xn1")
            xn18 = xn18p.tile([P, KC * N], F8, tag="xn18", name="xn18")
            qt = [qtp.tile([P, R], BF16, tag="qt", name=f"qt{i}") for i in range(KC)]
            kt = [ktp.tile([P, N], BF16, tag="kt", name=f"kt{i}") for i in range(KC)]
            vt = [vtp.tile([P, H * (D + 1)], BF16, tag="vt", name=f"vt{i}")
                  for i in range(NT)]
            wk8_sb = [wattn.tile([P, 2 * C], F8, tag="wk8", name=f"wk8_{j}", bufs=4)
                      for j in range(KC // 2)]
            msk_sb = [wattn.tile([P, R], BF16, tag="msk", name=f"msk{i}", bufs=NT)
                      for i in range(NT)]

            with tc.tile_pool(name="wqvp", bufs=8) as wqvp, \
                 tc.tile_pool(name="ln_ps", bufs=2, space="PSUM") as ln_ps, \
                 tc.tile_pool(name="ln_bc", bufs=2, space="PSUM") as ln_bc, \
                 tc.tile_pool(name="mm_ps", bufs=2, space="PSUM") as mm_ps:

                # --- x (bf16) loads first on the 2 fast queues: they gate
                # everything. xbt doubles as the attn-residual source later.
                xbt = [xsp.tile([P, 4 * 512], BF16, tag="xs", name=f"xb_{i}", bufs=4)
                       for i in range(4)]
                xengs = [nc.sync, nc.scalar, nc.sync, nc.scalar]
                for i in range(4):
                    xengs[i].dma_start(out=xbt[i][:],
                                       in_=io["xTb"][:, i * 2048:(i + 1) * 2048])

                # --- then weights / masks / biases in need order
                wq8_sb = [wqvp.tile([P, 2 * C], F8, tag="wq8", name=f"wq8_{j}", bufs=4)
                          for j in range(KC // 2)]
                wv_sb = [wqvp.tile([P, C], BF16, tag="wv", name=f"wv{i}", bufs=8)
                         for i in range(KC)]
                for j in range(KC // 2):
                    nc.gpsimd.dma_start(out=wq8_sb[j][:],
                                        in_=io["wq8"][:, j * 2 * C:(j + 1) * 2 * C])
                for k in range(KC):
                    eng = nc.gpsimd if k % 2 == 0 else nc.sync
                    eng.dma_start(out=wv_sb[k][:], in_=io["wv"][k * P:(k + 1) * P, :])
                nc.sync.dma_start(out=bq_sb[:], in_=bias_cols("bq", C))
                nc.sync.dma_start(out=bk_sb[:], in_=bias_cols("bk", C))
                nc.gpsimd.dma_start(out=bv_b[:],
                                    in_=bass.AP(tensor=io["bv"][:].tensor,
                                                offset=0, ap=[[0, P], [1, C]]))
                for j in range(KC // 2):
                    nc.gpsimd.dma_start(out=wk8_sb[j][:],
                                        in_=io["wk8"][:, j * 2 * C:(j + 1) * 2 * C])
                for t in range(NT):
                    eng = nc.gpsimd if t % 2 == 0 else nc.sync
                    eng.dma_start(out=msk_sb[t][:], in_=io["mskT"][t * P:(t + 1) * P, :])
                nc.sync.dma_start(out=sela[:], in_=io["sel2"][0:1, :])
                nc.sync.dma_start(out=selb[:], in_=io["sel2"][1:2, :])
                nc.sync.dma_start(out=bo_sb[:], in_=bias_cols("bo", C))
                nc.sync.dma_start(out=b1_sb[:], in_=bias_cols("b1", DFF))
                nc.sync.dma_start(out=b2_sb[:], in_=bias_cols("b2", C))

                wup_ct = [0]

                def warm(n_mm):
                    wup = mm_ps.tile([P, 512], F32, tag="mm",
                                     name=f"wup{wup_ct[0]}")
                    for i in range(n_mm):
                        nc.tensor.matmul(wup[:], wupt[:, 0:P], wupt[:],
                                         start=(i == 0), stop=(i == n_mm - 1))
                    ws_ = vecp.tile([1, 1], F32, tag="vec", name=f"wups{wup_ct[0]}")
                    nc.scalar.copy(ws_[:], wup[0:1, 0:1])
                    wup_ct[0] += 1

                warm(16)

                # --- LN1 over 2 chunks (bf16 stats via ones-matmul reductions)
                # all four stat rows share ONE psum bank: [s0|q0|s1|q1]
                inv_c = 1.0 / C
                ps_st0 = ln_ps.tile([1, 2 * 512], F32, tag="lnstat", name="ln1st0")
                ps_st1 = ln_ps.tile([1, 2 * 512], F32, tag="lnstat", name="ln1st1")

                def ln_chain(ps_s, ps_q, tag):
                    """[1,512] psum sums -> mu_bc/rstd_bc [P,512] bf16 SBUF."""
                    mu = vecp.tile([1, 512], F32, tag="vec", name=f"mu{tag}")
                    nc.vector.tensor_scalar_mul(mu[:], ps_s, inv_c)
                    msq = vecp.tile([1, 512], F32, tag="vec", name=f"msq{tag}")
                    nc.vector.tensor_mul(msq[:], mu[:], mu[:])
                    varr = vecp.tile([1, 512], F32, tag="vec", name=f"var{tag}")
                    nc.vector.scalar_tensor_tensor(varr[:], ps_q, inv_c, msq[:],
                                                   op0=ALU.mult, op1=ALU.subtract)
                    std = vecp.tile([1, 512], F32, tag="vec", name=f"std{tag}")
                    nc.scalar.activation(std[:], varr[:], AF.Sqrt, bias=eps_sb[:])
                    rstd = vecp.tile([1, 512], F32, tag="vec", name=f"rstd{tag}")
                    nc.vector.reciprocal_approx_fast(out=rstd[:], in_=std[:])
                    mu_b = vecp.tile([1, 512], BF16, tag="vb16", name=f"mub{tag}", bufs=2)
                    rstd_b = vecp.tile([1, 512], BF16, tag="vb16", name=f"rsb{tag}", bufs=2)
                    nc.scalar.copy(mu_b[:], mu[:])
                    nc.vector.tensor_copy(out=rstd_b[:], in_=rstd[:])
                    return mu_b, rstd_b

                def ln_bcast(mu_b, rstd_b, tag):
                    ps_mu = ln_bc.tile([P, 512], F32, tag="lnbc", name=f"bmu{tag}")
                    ps_rstd = ln_bc.tile([P, 512], F32, tag="lnbc", name=f"brs{tag}")
                    nc.tensor.matmul(ps_mu[:], ones_cb[:], mu_b[:], start=True, stop=True)
                    nc.tensor.matmul(ps_rstd[:], ones_cb[:], rstd_b[:], start=True, stop=True)
                    mu_bc = bcp.tile([P, 512], BF16, tag="bc", name=f"mubc{tag}", bufs=2)
                    rstd_bc = bcp.tile([P, 512], BF16, tag="bc", name=f"rsbc{tag}", bufs=2)
                    nc.vector.tensor_copy(out=mu_bc[:], in_=ps_mu[:])
                    nc.vector.tensor_copy(out=rstd_bc[:], in_=ps_rstd[:])
                    return mu_bc, rstd_bc

                def ln1_norm(chunk, mu_bc, rstd_bc):
                    # chunk-block layout keeps writes contiguous:
                    # col = chunk*4096 + k*512 + tok
                    for half in range(2):
                        xb = xbt[chunk * 2 + half]
                        t1 = t1p.tile([P, 2048], BF16, tag="t1",
                                      name=f"t1_{chunk}_{half}")
                        nc.vector.tensor_tensor(t1[:].rearrange("p (b w) -> p b w", w=512),
                                                xb[:].rearrange("p (b w) -> p b w", w=512),
                                                _rep(mu_bc[:], 4, 512), op=ALU.subtract)
                        obase = chunk * 4096 + half * 2048
                        dst = xn1[:, obase:obase + 2048]
                        nc.vector.tensor_tensor(
                            dst.rearrange("p (b w) -> p b w", w=512),
                            t1[:].rearrange("p (b w) -> p b w", w=512),
                            _rep(rstd_bc[:], 4, 512), op=ALU.mult)
                        # fp8 copy for Q/K DoubleRow rhs (contiguous, on scalar)
                        nc.scalar.copy(xn18[:, obase:obase + 2048], dst)

                # stats for BOTH chunks back-to-back on PE
                for chunk in range(2):
                    sqc = [None, None]
                    for half in range(2):
                        xb = xbt[chunk * 2 + half]
                        sqc[half] = sqp.tile([P, 2048], BF16, tag="sq",
                                             name=f"sq1_{chunk}_{half}", bufs=2)
                        nc.vector.tensor_mul(sqc[half][:], xb[:], xb[:])
                    ps_c = ps_st0 if chunk == 0 else ps_st1
                    ps_s = ps_c[:, 0:512]
                    ps_q = ps_c[:, 512:1024]
                    for k in range(KC):
                        xc = xbt[chunk * 2 + k // 4][:, (k % 4) * 512:(k % 4 + 1) * 512]
                        sq = sqc[k // 4][:, (k % 4) * 512:(k % 4 + 1) * 512]
                        nc.tensor.matmul(ps_s, ones_kb[:], xc,
                                         start=(k == 0), stop=(k == KC - 1))
                        nc.tensor.matmul(ps_q, ones_kb[:], sq,
                                         start=(k == 0), stop=(k == KC - 1))
                warm(12)
                mu_b0, rstd_b0 = ln_chain(ps_st0[:, 0:512], ps_st0[:, 512:1024], "c0")
                mu_bc0, rstd_bc0 = ln_bcast(mu_b0, rstd_b0, "c0")
                warm(12)
                ln1_norm(0, mu_bc0, rstd_bc0)

                # ---- V projection first two token tiles (needs only xn1 chunk 0)
                def emit_v(t):
                    vre = vt[t][:].rearrange("p (h j) -> p h j", j=D + 1)
                    nc.vector.memset(vre[:, :, D:D + 1], 1.0)
                    for half in range(2):
                        ps = mm_ps.tile([P, 512], F32, tag="mm")
                        for k in range(KC):
                            xb = (t // 4) * 4096 + k * 512 + (t % 4) * P
                            nc.tensor.matmul(ps[:], xn1[:, xb:xb + P],
                                             wv_sb[k][:, half * 512:(half + 1) * 512],
                                             start=(k == 0), stop=(k == KC - 1))
                        nc.vector.tensor_tensor(
                            vre[:, 8 * half:8 * (half + 1), 0:D],
                            ps[:].rearrange("p (h j) -> p h j", j=D),
                            bv_b[:, half * 512:(half + 1) * 512].rearrange(
                                "p (h j) -> p h j", j=D),
                            op=ALU.add)

                emit_v(0)
                emit_v(1)
                emit_v(2)
                emit_v(3)

                # chunk-1 chain overlaps V0-3 PE work; its broadcast matmuls
                # land in the PE FIFO after V0-3
                mu_b1, rstd_b1 = ln_chain(ps_st1[:, 0:512], ps_st1[:, 512:1024], "c1")
                mu_bc1, rstd_bc1 = ln_bcast(mu_b1, rstd_b1, "c1")

                # ---- Q projection (own tokens only), fp8 DoubleRow
                for m in range(KC):
                    ps = mm_ps.tile([P, 512], F32, tag="mm")
                    for j in range(KC // 2):
                        w8r = wq8_sb[j][:].rearrange("p (t m) -> p t m", t=2)
                        x8r = xn18[:, j * 1024:(j + 1) * 1024].rearrange(
                            "p (t n) -> p t n", t=2)
                        nc.tensor.matmul(ps[:], w8r[:, :, m * P:(m + 1) * P],
                                         x8r[:],
                                         start=(j == 0), stop=(j == KC // 2 - 1),
                                         perf_mode=DR)
                    nc.scalar.activation(qt[m][:], ps[:], AF.Identity,
                                         scale=IWS * float(D) ** -0.5,
                                         bias=bq_sb[:, m:m + 1])

                ln1_norm(1, mu_bc1, rstd_bc1)
                for t in range(4, NT):
                    emit_v(t)

            # ---- attention pair pipeline (K projection pipelined one ahead)
            with tc.tile_pool(name="simps", bufs=2, space="PSUM") as simps, \
                 tc.tile_pool(name="ops", bufs=1, space="PSUM") as ops_, \
                 tc.tile_pool(name="mmb", bufs=2, space="PSUM") as mmb, \
                 tc.tile_pool(name="rbps", bufs=1, space="PSUM") as rbps, \
                 tc.tile_pool(name="a2p", bufs=10) as a2p, \
                 tc.tile_pool(name="recp", bufs=2) as recp, \
                 tc.tile_pool(name="smr", bufs=2) as smr:

                # prefetch wo during attention (gpsimd queue — sync is busy)
                for k in range(KC):
                    nc.gpsimd.dma_start(out=wo_sb[k][:], in_=io["wo"][k * P:(k + 1) * P, :])

                a_tiles = {}
                sums = {}

                def emit_k(p):
                    for nn_ in range(2):
                        ps = mmb.tile([P, 512], F32, tag="mm", name=f"kp{p}_{nn_}")
                        for j in range(KC // 2):
                            w8r = wk8_sb[j][:].rearrange("p (t m) -> p t m", t=2)
                            xb = nn_ * 4096 + j * 1024
                            x8r = xn18[:, xb:xb + 1024].rearrange(
                                "p (t n) -> p t n", t=2)
                            nc.tensor.matmul(ps[:], w8r[:, :, p * P:(p + 1) * P],
                                             x8r[:],
                                             start=(j == 0), stop=(j == KC // 2 - 1),
                                             perf_mode=DR)
                        nc.vector.scalar_tensor_tensor(
                            kt[p][:, nn_ * 512:(nn_ + 1) * 512], ps[:], IWS,
                            _bc(bk_sb[:, p:p + 1], 512), op0=ALU.mult, op1=ALU.add)

                def emit_qk(p):
                    kth0 = kt[p][0:D, :]
                    kth1 = kt[p][D:2 * D, :]
                    qth0 = qt[p][0:D, :]
                    qth1 = qt[p][D:2 * D, :]
                    for tk in range(NT):
                        ps2 = simps.tile([P, 2 * R], F32, tag="sim", name=f"sim{p}_{tk}")
                        nc.tensor.matmul(ps2[:, 0:R], kth0[:, tk * P:(tk + 1) * P], qth0[:],
                                         start=True, stop=True)
                        nc.tensor.matmul(ps2[:, R:2 * R], kth1[:, tk * P:(tk + 1) * P], qth1[:],
                                         start=True, stop=True)
                        a2 = a2p.tile([P, 2 * R], BF16, tag="a", name=f"a{p}_{tk}")
                        nc.scalar.activation(a2[:], ps2[:], AF.Exp)
                        mrep = bass.AP(tensor=msk_sb[tk][:].tensor,
                                       offset=msk_sb[tk][:].offset,
                                       ap=[msk_sb[tk][:].ap[0], [0, 2], [1, R]])
                        nc.vector.tensor_tensor(
                            a2[:].rearrange("p (h j) -> p h j", j=R),
                            a2[:].rearrange("p (h j) -> p h j", j=R), mrep, op=ALU.mult)
                        a_tiles[(p, tk)] = a2

                def emit_o(p):
                    s0 = smr.tile([1, R], F32R, tag="s0", name=f"s0_{p}")
                    s1 = smr.tile([1, R], F32R, tag="s1", name=f"s1_{p}")
                    sums[p] = (s0, s1)
                    for hh in range(2):
                        h = 2 * p + hh
                        ps_o = ops_.tile([D + 1, R], F32, tag="o", name=f"o{h}")
                        for tk in range(NT):
                            vre = vt[tk][:].rearrange("p (h j) -> p h j", j=D + 1)
                            nc.tensor.matmul(ps_o[:], vre[:, h, 0:D + 1],
                                             a_tiles[(p, tk)][:, hh * R:(hh + 1) * R],
                                             start=(tk == 0), stop=(tk == NT - 1))
                        dst = s0 if hh == 0 else s1
                        nc.vector.tensor_copy(out=dst[0:1, :], in_=ps_o[D:D + 1, :])
                        nc.vector.tensor_copy(out=ot[p][hh * D:(hh + 1) * D, :],
                                              in_=ps_o[0:D, :])
                    for tk in range(NT):
                        del a_tiles[(p, tk)]

                def emit_norm(p):
                    s0, s1 = sums.pop(p)
                    ps_rb = rbps.tile([P, R], F32, tag="rb", name=f"rb{p}")
                    nc.tensor.matmul(ps_rb[:], sela[:], s0[0:1, :],
                                     start=True, stop=False)
                    nc.tensor.matmul(ps_rb[:], selb[:], s1[0:1, :],
                                     start=False, stop=True)
                    rec_sb = recp.tile([P, R], F32, tag="rec", name=f"rec{p}")
                    nc.vector.reciprocal_approx_fast(out=rec_sb[:], in_=ps_rb[:])
                    nc.vector.tensor_tensor(ot[p][:], ot[p][:], rec_sb[:], op=ALU.mult)

                emit_k(0)
                for p in range(H // 2):
                    emit_qk(p)
                    if p < H // 2 - 1:
                        emit_k(p + 1)
                    emit_o(p)
                    emit_norm(p)

                # preload sqrt+gelu activation tables right after the last exp
                # (input dep on ot[7] pins these behind the attention tail)
                tl1 = vecp.tile([1, 1], F32, tag="vec", name="tl1")
                tl2 = vecp.tile([1, 1], F32, tag="vec", name="tl2")
                nc.scalar.activation(tl1[:], ot[7][0:1, 0:1], AF.Sqrt)
                nc.scalar.activation(tl2[:], tl1[:], AF.Gelu_apprx_tanh)

        # ================= attn out projection + residual + LN2 stats =================
        # w18 loads during this phase into a late-opened pool
        w18p = ctx.enter_context(tc.tile_pool(name="w18p", bufs=KC // 2))
        w18_sb = [w18p.tile([P, 2 * DFF], F8, tag="w18", name=f"w18_{j}")
                  for j in range(KC // 2)]
        for j in range(KC // 2):
            eng = nc.sync if j % 2 == 0 else nc.gpsimd
            eng.dma_start(out=w18_sb[j][:],
                          in_=io["w18"][:, j * 2 * DFF:(j + 1) * 2 * DFF])

        inv_c = 1.0 / C
        with tc.tile_pool(name="ln_ps2", bufs=2, space="PSUM") as ln_ps, \
             tc.tile_pool(name="ln_bc2", bufs=2, space="PSUM") as ln_bc, \
             tc.tile_pool(name="mm_ps2", bufs=3, space="PSUM") as mm_ps:
            ps_s = ln_ps.tile([1, 512], F32, tag="lnstat", name="ln2s")
            ps_q = ln_ps.tile([1, 512], F32, tag="lnstat", name="ln2q")
            sq2h = [sqp.tile([P, 4 * R], BF16, tag="sq", name=f"sq2h{i}", bufs=2)
                    for i in range(2)]

            def oproj(m):
                ps = mm_ps.tile([P, 512], F32, tag="mm")
                for k in range(KC):
                    nc.tensor.matmul(ps[:], wo_sb[k][:, m * P:(m + 1) * P], ot[k][:],
                                     start=(k == 0), stop=(k == KC - 1))
                # residual x (bf16) is still resident in the xbt input tiles
                xr = xbt[m // 4][:, (m % 4) * 512:(m % 4 + 1) * 512]
                nc.vector.scalar_tensor_tensor(x2[:, m * R:(m + 1) * R], ps[:],
                                               bo_sb[:, m:m + 1], xr,
                                               op0=ALU.add, op1=ALU.add)

            def ln2_stats(k):
                nc.tensor.matmul(ps_s[:], ones_kb[:], x2[:, k * R:(k + 1) * R],
                                 start=(k == 0), stop=(k == KC - 1))
                nc.tensor.matmul(ps_q[:], ones_kb[:],
                                 sq2h[k // 4][:, (k % 4) * R:(k % 4 + 1) * R],
                                 start=(k == 0), stop=(k == KC - 1))

            wup_ct2 = [0]

            def warm2(n_mm):
                wup = ln_bc.tile([P, 512], F32, tag="lnbc", name=f"w2u{wup_ct2[0]}")
                for i in range(n_mm):
                    nc.tensor.matmul(wup[:], wupt[:, 0:P], wupt[:],
                                     start=(i == 0), stop=(i == n_mm - 1))
                wsb = vecp.tile([1, 1], F32, tag="vec", name=f"w2us{wup_ct2[0]}")
                nc.scalar.copy(wsb[:], wup[0:1, 0:1])
                wup_ct2[0] += 1

            for m in range(4):
                oproj(m)
            nc.vector.tensor_mul(sq2h[0][:], x2[:, 0:4 * R], x2[:, 0:4 * R])
            for m in range(4, KC):
                oproj(m)
                ln2_stats(m - 4)
            nc.vector.tensor_mul(sq2h[1][:], x2[:, 4 * R:8 * R], x2[:, 4 * R:8 * R])
            warm2(16)
            for k in range(4, KC):
                ln2_stats(k)
            warm2(14)

            mu_bc, rstd_bc = None, None
            mu = vecp.tile([1, 512], F32, tag="vec", name="mu_l2")
            nc.vector.tensor_scalar_mul(mu[:], ps_s[:], inv_c)
            msq = vecp.tile([1, 512], F32, tag="vec", name="msq_l2")
            nc.vector.tensor_mul(msq[:], mu[:], mu[:])
            varr = vecp.tile([1, 512], F32, tag="vec", name="var_l2")
            nc.vector.scalar_tensor_tensor(varr[:], ps_q[:], inv_c, msq[:],
                                           op0=ALU.mult, op1=ALU.subtract)
            std = vecp.tile([1, 512], F32, tag="vec", name="std_l2")
            nc.scalar.activation(std[:], varr[:], AF.Sqrt, bias=eps_sb[:])
            rstd = vecp.tile([1, 512], F32, tag="vec", name="rstd_l2")
            nc.vector.reciprocal_approx_fast(out=rstd[:], in_=std[:])
            mu_b = vecp.tile([1, 512], BF16, tag="vb16", name="mub_l2", bufs=2)
            rstd_b = vecp.tile([1, 512], BF16, tag="vb16", name="rsb_l2", bufs=2)
            nc.gpsimd.tensor_copy(out=mu_b[:], in_=mu[:])
            nc.vector.tensor_copy(out=rstd_b[:], in_=rstd[:])
            ps_mu = ln_bc.tile([P, 512], F32, tag="lnbc", name="bmu_l2")
            ps_rstd = ln_bc.tile([P, 512], F32, tag="lnbc", name="brs_l2")
            nc.tensor.matmul(ps_mu[:], ones_cb[:], mu_b[:], start=True, stop=True)
            nc.tensor.matmul(ps_rstd[:], ones_cb[:], rstd_b[:], start=True, stop=True)
            mu_bc = bcp.tile([P, 512], BF16, tag="bc", name="mubc_l2", bufs=2)
            rstd_bc = bcp.tile([P, 512], BF16, tag="bc", name="rsbc_l2", bufs=2)
            nc.vector.tensor_copy(out=mu_bc[:], in_=ps_mu[:])
            nc.vector.tensor_copy(out=rstd_bc[:], in_=ps_rstd[:])
            # keep PE warm while the normalize runs on the vector engine
            warm2(12)
            # normalize in two wide bf16 halves -> fp8 (k-contiguous layout)
            for hf in range(2):
                t1 = t1p.tile([P, 4 * R], BF16, tag="t1", name=f"t1l2_{hf}")
                nc.vector.tensor_tensor(
                    t1[:].rearrange("p (b w) -> p b w", w=R),
                    x2[:, hf * 4 * R:(hf + 1) * 4 * R].rearrange("p (b w) -> p b w", w=R),
                    _rep(mu_bc[:], 4, R), op=ALU.subtract)
                nc.vector.tensor_tensor(
                    xn28[:, hf * 4 * R:(hf + 1) * 4 * R].rearrange("p (b w) -> p b w", w=R),
                    t1[:].rearrange("p (b w) -> p b w", w=R),
                    _rep(rstd_bc[:], 4, R), op=ALU.mult)

        # ================= MLP =================
        h1p = ctx.enter_context(tc.tile_pool(name="h1p", bufs=DFF // P))
        h1 = [h1p.tile([P, R], BF16, tag="h1", name=f"h1_{i}") for i in range(DFF // P)]
        with tc.tile_pool(name="w2p", bufs=6) as w2p, \
             tc.tile_pool(name="mm_ps3", bufs=3, space="PSUM") as mm_ps, \
             tc.tile_pool(name="fc2ps", bufs=2, space="PSUM") as fc2ps:
            # w2 packed as (mg, k) blocks of [128, 512]; 8 DMAs of [128, 4096]
            w2_sb = [w2p.tile([P, 4096], BF16, tag="w2s", name=f"w2_{g}", bufs=6)
                     for g in range(KC)]
            for g in range(KC):
                nc.gpsimd.dma_start(out=w2_sb[g][:],
                                    in_=io["w2"][:, g * 4096:(g + 1) * 4096])

            def w2ap(mg, k, om):
                g = mg * 4 + k // 8
                return w2_sb[g][:, (k % 8) * 512 + om * P:(k % 8) * 512 + (om + 1) * P]

            # fc1 (fp8 DoubleRow) + gelu
            for m in range(DFF // P):
                ps = mm_ps.tile([P, 512], F32, tag="mm")
                for j in range(KC // 2):
                    w8r = w18_sb[j][:].rearrange("p (t m) -> p t m", t=2)
                    x8r = xn28[:, j * 2 * R:(j + 1) * 2 * R].rearrange(
                        "p (t n) -> p t n", t=2)
                    nc.tensor.matmul(ps[:], w8r[:, :, m * P:(m + 1) * P], x8r,
                                     start=(j == 0), stop=(j == KC // 2 - 1),
                                     perf_mode=DR)
                nc.scalar.activation(h1[m][:], ps[:], AF.Gelu_apprx_tanh, scale=IWS,
                                     bias=b1_sb[:, m:m + 1])

            # fc2 (bf16) m-outer: epilogue + output DMA staggered per m-tile
            for om in range(KC):
                ps = fc2ps.tile([P, 512], F32, tag="fc2", name=f"fc2ps{om}")
                for k in range(DFF // P):
                    nc.tensor.matmul(ps[:], w2ap(om // 4, k, om % 4),
                                     h1[k][:], start=(k == 0),
                                     stop=(k == DFF // P - 1))
                y_sb = yp.tile([P, R], F32, tag="y", name=f"y{om}")
                nc.vector.scalar_tensor_tensor(y_sb[:], ps[:], b2_sb[:, om:om + 1],
                                               x2[:, om * R:(om + 1) * R],
                                               op0=ALU.add, op1=ALU.add)
                eng = nc.sync if om % 2 == 0 else nc.gpsimd
                eng.dma_start(out=io["yT"][om * P:(om + 1) * P, :], in_=y_sb[:])

    if not nc.is_finalized():
        nc.finalize()
    return nc


def _get_nc():
    if "nc" not in _CACHE:
        _CACHE["nc"] = _build()
    return _CACHE["nc"]


def _pack_pairs(w):
    """[K, M] -> [128, (K/256)*2*M]: per pair j, [W_{2j} | W_{2j+1}]."""
    K_, M_ = w.shape
    nj = K_ // (2 * P)
    out = np.empty((P, nj * 2 * M_), dtype=w.dtype)
    for j in range(nj):
        out[:, j * 2 * M_:j * 2 * M_ + M_] = w[2 * j * P:(2 * j + 1) * P, :]
        out[:, j * 2 * M_ + M_:(j + 1) * 2 * M_] = w[(2 * j + 1) * P:(2 * j + 2) * P, :]
    return out


def _prep_in_maps(inputs):
    x = np.asarray(inputs["x"], dtype=np.float32)
    mask = np.asarray(inputs["mask"])
    scale = float(D) ** -0.5
    wq = np.asarray(inputs["wq"], np.float32)
    bq = (np.asarray(inputs["bq"], np.float32) * scale).astype(np.float32)
    wkv = np.asarray(inputs["wkv"], np.float32)
    bkv = np.asarray(inputs["bkv"], np.float32)
    wk = np.ascontiguousarray(wkv[:, :C])
    wv = np.ascontiguousarray(wkv[:, C:]).astype(BFNP)
    bk = np.ascontiguousarray(bkv[:C]).astype(np.float32)
    bv = np.ascontiguousarray(bkv[C:]).astype(np.float32)
    wo = np.asarray(inputs["wo"], np.float32).astype(BFNP)
    bo = np.asarray(inputs["bo"], np.float32)
    w1 = np.asarray(inputs["w1"], np.float32)
    b1 = np.asarray(inputs["b1"], np.float32)
    w2r = np.asarray(inputs["w2"], np.float32).astype(BFNP)
    # pack w2 as (mg, k) blocks: w2pk[p, mg*16384 + k*512 + j] = w2[k*128+p, mg*512+j]
    w2 = np.ascontiguousarray(
        w2r.reshape(DFF // P, P, 2, 512).transpose(1, 2, 0, 3).reshape(P, DFF // P * C))
    b2 = np.asarray(inputs["b2"], np.float32)
    mask01 = mask.astype(np.float32)

    # fp8 weights, pre-scaled by WS (epilogues rescale by 1/WS); q additionally
    # folds d^-0.5 into its activation scale, and bq is pre-scaled by d^-0.5
    wq8 = _pack_pairs(np.clip(wq * WS, -240, 240).astype(F8NP))
    wk8 = _pack_pairs(np.clip(wk * WS, -240, 240).astype(F8NP))
    w18 = _pack_pairs(np.clip(w1 * WS, -240, 240).astype(F8NP))

    sel2 = np.zeros((2, P), dtype=np.float32)
    sel2[0, 0:D] = 1.0
    sel2[1, D:2 * D] = 1.0
    shared = dict(wq8=wq8, wk8=wk8, w18=w18, w2=w2, wv=wv, wo=wo,
                  bq=bq, bk=bk, bv=bv, bo=bo, b1=b1, b2=b2, sel2=sel2)
    in_maps = []
    for c in range(8):
        b = c // 2
        rh = c % 2
        own = np.arange(rh * R, rh * R + R)
        oth = np.arange((1 - rh) * R, (1 - rh) * R + R)
        perm = np.concatenate([own, oth])
        xT = np.ascontiguousarray(x[b].T[:, perm])
        mskT = np.ascontiguousarray(mask01[np.ix_(own, perm)].T).astype(BFNP)
        m = dict(shared)
        # pack x (bf16): blocks ordered (chunk, k) of [128, 512]
        xb = xT.astype(BFNP).reshape(KC, P, 2, 512).transpose(1, 2, 0, 3).reshape(P, 2 * KC * 512)
        m["xTb"] = np.ascontiguousarray(xb)
        m["mskT"] = mskT
        in_maps.append(m)
    return in_maps


def _assemble(results):
    out = np.empty((B, N, C), dtype=np.float32)
    for c in range(8):
        b = c // 2
        rh = c % 2
        out[b, rh * R:(rh + 1) * R, :] = results[c]["yT"].T
    return out


def run(inputs, trace=False):
    nc = _get_nc()
    in_maps = _prep_in_maps(inputs)
    res = run_bass_kernel_spmd(nc, in_maps, core_ids=list(range(8)), trace=trace)
    return _assemble(res.results), res


def kernel(**inputs):
    out, _ = run(inputs, trace=False)
    return out
